# revision 1
# baseline (speedup 1.0000x reference)
"""Distributed causal multi-head attention (RoPE) for 8 TRN2 NeuronCores.

Problem: B=4, S=2048, D=2048, H=16 heads, DH=128.
Sharding: 2D — data-parallel over the 4 batches x tensor-parallel over 2
head-groups of 8 heads (Megatron-style: Wqkv column-sharded per head
group, Wo row-sharded).  Core c handles batch c//2, head group c%2.
Each core returns a partial output projection [S, D] in bf16; the host
sums the two group partials per batch (the "all-reduce") and stacks
batches.

All operands are bf16 (the PE runs bf16 at the same 1 cycle/row as
fp32r but with half the SBUF/DMA footprint and LDWEIGHTS fully hidden
under the moving stream), accumulation in fp32 PSUM.  Everything stays
SBUF-resident — no DRAM spill of Q/K/V.  fp8 was measured and rejected:
attention output is a weighted mean, so per-element quantization error
lands ~1:1 in the output; e4m3 anywhere in the signal path costs 2-4%
vs the 2e-2 budget.

Per-core pipeline (~617us; PE sustains ~2.26GHz, 94% of peak):
  stage 1: QKV projection, streamed in consumption order on the single
           fast SP hardware DMA queue (x chunk-major 16KB-run DMAs; DMAs
           triggered from ACT/GPSIMD queues measurably hurt).  RoPE is
           fused into the PSUM eviction: ACT evicts psum to bf16 twice
           (straight + partition-rotated halves, the rotate_half swap),
           then three 2x-mode DVE ops (mul cos, mul sign-folded sin,
           add) write the resident per-head q/k tiles [dh, tok].  V
           lands in vres [tok, kt, head*129] with a ones column
           (gpsimd memset) so PV yields the softmax denominator free.
  stage 2: query-chunk-outer, head-inner.  Per (qr, h): scoresT[k,q] =
           K-tile.T x Q (causally trimmed), exp via ACT (scale fused)
           into bf16 tiles, triangular mask multiply on the diagonal
           blocks only, PV via bf16 matmuls with the fused ones column,
           reciprocal scale + PE transpose (bf16 identity) into
           resident zT tiles.  QK of job j+1 is emitted before PV of
           job j (PE is in-order; this hides the ACT exp latency) and
           transposes are delayed one step to hide the DVE epilogue.
  stage 3: out = sum_h zT_h.T x WoT_h with Wo fully SBUF-resident,
           interleaved under stage 2: the 16 output chunks of
           query-chunk qr are emitted two-per-head during qr+1, filling
           PE time while ACT works on exps.
"""

import sys

if '/opt/trn_rl_repo' not in sys.path:
    sys.path.insert(0, '/opt/trn_rl_repo')

import math

import ml_dtypes
import numpy as np

B, S, D, H, DH = 4, 2048, 2048, 16, 128
BASE = 10000.0
P = 128
NT = S // P          # 16 token tiles of 128
NC512 = S // 512     # 4 token chunks of 512
NDM = D // P         # 16 d_model chunks
HG = 8               # heads per group
VW = P + 1           # v block width per head (128 + ones column)
SCALE = 1.0 / math.sqrt(DH)

_CACHE = {}


def _build_program():
    import concourse.bacc as bacc
    import concourse.mybir as mybir
    from concourse.tile import TileContext
    from concourse.masks import make_identity

    F32 = mybir.dt.float32
    BF16 = mybir.dt.bfloat16
    EXP = mybir.ActivationFunctionType.Exp

    nc = bacc.Bacc('TRN2', target_bir_lowering=False, debug=False, num_devices=8)

    # ---- DRAM I/O ----
    xT = nc.dram_tensor('xT', [NC512, P, NDM, 512], BF16, kind='ExternalInput').ap()
    wqkT = nc.dram_tensor('wqkT', [2 * HG, P, NDM, P], BF16, kind='ExternalInput').ap()
    wvT = nc.dram_tensor('wvT', [4, P, NDM, 256], BF16, kind='ExternalInput').ap()
    woT = nc.dram_tensor('woT', [NC512, P, HG, 512], BF16, kind='ExternalInput').ap()
    cosT = nc.dram_tensor('cosT', [P, S], BF16, kind='ExternalInput').ap()
    sinP = nc.dram_tensor('sinP', [P, S], BF16, kind='ExternalInput').ap()
    maskT = nc.dram_tensor('maskT', [P, P], BF16, kind='ExternalInput').ap()
    out = nc.dram_tensor('out', [NT, P, D], BF16, kind='ExternalOutput').ap()

    with TileContext(nc) as tc:
        with tc.tile_pool(name='res', bufs=1) as rpool:
            msk = rpool.tile([P, P], BF16)
            identb = rpool.tile([P, P], BF16)
            qres = [rpool.tile([P, S], BF16, name=f'q{h}') for h in range(HG)]
            kres = [rpool.tile([P, S], BF16, name=f'k{h}') for h in range(HG)]
            vres = rpool.tile([P, NT, HG * VW], BF16)


            # ================= stage 1: QKV projection =================
            with tc.tile_pool(name='s1x', bufs=1) as xpool, \
                 tc.tile_pool(name='s1w', bufs=2) as wpool, \
                 tc.tile_pool(name='s1e', bufs=3) as epool, \
                 tc.tile_pool(name='s1p', bufs=4, space='PSUM') as qpp, \
                 tc.tile_pool(name='s1pv', bufs=4, space='PSUM') as vpp:
                xsb = xpool.tile([P, NC512, NDM, 512], BF16)
                cos_sb = xpool.tile([P, S], BF16)
                sin_sb = xpool.tile([P, S], BF16)
                # Early input feed, consumption-ordered on the single fast
                # hardware DMA queue (SP/sync).  DMAs triggered from other
                # engines (ACT hw queue, GPSIMD sw queue) measurably hurt:
                # they block that engine's sequencer until the transfer
                # lands, and sw DGE is slow.
                nc.sync.dma_start(xsb[:, 0], xT[0])
                w0 = wpool.tile([P, NDM, P], BF16, tag='w', name='w0')
                nc.sync.dma_start(w0[:], wqkT[0])
                nc.sync.dma_start(xsb[:, 1], xT[1])
                nc.sync.dma_start(cos_sb[:], cosT[:])
                nc.sync.dma_start(sin_sb[:], sinP[:])
                for tcn in range(2, NC512):
                    nc.sync.dma_start(xsb[:, tcn], xT[tcn])
                nc.sync.dma_start(msk[:], maskT[:])
                make_identity(nc, identb[:])
                for h in range(HG):
                    nc.gpsimd.memset(vres[:, :, h * VW + P:h * VW + P + 1], 1.0)

                for fb in range(2 * HG):
                    if fb == 0:
                        w = w0
                    else:
                        w = wpool.tile([P, NDM, P], BF16, tag='w', name=f'w{fb}')
                        nc.sync.dma_start(w[:], wqkT[fb])
                    dest = qres[fb] if fb < HG else kres[fb - HG]
                    for tcn in range(NC512):
                        ts = slice(tcn * 512, tcn * 512 + 512)
                        ps = qpp.tile([P, 512], F32, tag='pqk', name=f'pqk_{fb}_{tcn}')
                        for o in range(NDM):
                            nc.tensor.matmul(ps[:], w[:, o, :], xsb[:, tcn, o, :],
                                             start=(o == 0), stop=(o == NDM - 1))
                        # RoPE fused eviction (sign folded into sinP).  The
                        # rotate_half partition swap happens in the ACT psum
                        # eviction so every DVE op is same-base-partition
                        # bf16 (2x mode).
                        psb = epool.tile([P, 512], BF16, tag='psb', name=f'psb_{fb}_{tcn}')
                        nc.scalar.copy(psb[:], ps[:])
                        psr = epool.tile([P, 512], BF16, tag='psr', name=f'psr_{fb}_{tcn}')
                        nc.scalar.copy(psr[0:64, :], ps[64:128, :])
                        nc.scalar.copy(psr[64:128, :], ps[0:64, :])
                        t1 = epool.tile([P, 512], BF16, tag='t1', name=f't1_{fb}_{tcn}')
                        t2 = epool.tile([P, 512], BF16, tag='t2', name=f't2_{fb}_{tcn}')
                        nc.vector.tensor_mul(t1[:], psb[:], cos_sb[:, ts])
                        nc.vector.tensor_mul(t2[:], psr[:], sin_sb[:, ts])
                        nc.vector.tensor_add(dest[:, ts], t1[:], t2[:])

                # --- V blocks, token-major ---
                for vc in range(4):
                    wv = wpool.tile([P, NDM, 256], BF16, tag='wv', name=f'wv{vc}')
                    nc.sync.dma_start(wv[:], wvT[vc])
                    for tt in range(NT):
                        psv = vpp.tile([P, 256], F32, tag='pv', name=f'pv_{vc}_{tt}')
                        for o in range(NDM):
                            nc.tensor.matmul(psv[:],
                                             xsb[:, tt // 4, o,
                                                 (tt % 4) * P:(tt % 4 + 1) * P],
                                             wv[:, o, :],
                                             start=(o == 0), stop=(o == NDM - 1))
                        for j in range(2):
                            hv = 2 * vc + j
                            nc.scalar.copy(vres[:, tt, hv * VW:hv * VW + P],
                                           psv[:, j * P:(j + 1) * P])

            # ============ stage 2 + interleaved stage 3 ============
            with tc.tile_pool(name='res2', bufs=1) as r2pool, \
                 tc.tile_pool(name='s2st', bufs=2) as stpool, \
                 tc.tile_pool(name='s2z', bufs=4) as zpool, \
                 tc.tile_pool(name='s2os', bufs=4) as ospool, \
                 tc.tile_pool(name='s2p', bufs=3, space='PSUM') as spp, \
                 tc.tile_pool(name='s2pz', bufs=2, space='PSUM') as zpp, \
                 tc.tile_pool(name='s2pt', bufs=1, space='PSUM') as tpp, \
                 tc.tile_pool(name='s3p', bufs=2, space='PSUM') as opp:

                zres = [r2pool.tile([P, S], BF16, name=f'z{h}') for h in range(HG)]
                wo_sb = r2pool.tile([P, NC512, HG, 512], BF16)
                for ec in range(NC512):
                    nc.sync.dma_start(wo_sb[:, ec], woT[ec])
                st = {}
                pend_t = []

                def flush_t():
                    ph, pqa, pzsb = pend_t.pop(0)
                    ztp = tpp.tile([P, P], BF16, tag='ztp', name=f'ztp_{ph}_{pqa}')
                    nc.tensor.transpose(ztp[:], pzsb[:], identb[:])
                    if pqa % 2 == 0:
                        nc.vector.tensor_copy(zres[ph][:, pqa * P:(pqa + 1) * P], ztp[:])
                    else:
                        nc.scalar.copy(zres[ph][:, pqa * P:(pqa + 1) * P], ztp[:])

                def emit_qk(qr, h):
                    base = qr * 512
                    tiles = []
                    for kt in range(4 * qr + 4):
                        d = kt - 4 * qr
                        off = 0 if d < 0 else P * d   # causal trim
                        sps = spp.tile([P, 512], F32, tag='sps',
                                       name=f'sps_{qr}_{h}_{kt}')
                        nc.tensor.matmul(sps[:, off:512],
                                         kres[h][:, kt * P:(kt + 1) * P],
                                         qres[h][:, base + off:base + 512],
                                         start=True, stop=True)
                        stt = stpool.tile([P, 512], BF16, tag=f'st{kt}',
                                          name=f'st_{qr}_{h}_{kt}')
                        nc.scalar.activation(stt[:, off:512], sps[:, off:512],
                                             EXP, scale=SCALE)
                        if d >= 0:
                            # triangular mask on the diagonal 128-block only
                            nc.vector.tensor_mul(stt[:, off:off + P],
                                                 stt[:, off:off + P], msk[:])
                        tiles.append(stt)
                    st[(qr, h)] = tiles

                def emit_pv(qr, h):
                    tiles = st.pop((qr, h))
                    for qs in range(4):
                        qa = 4 * qr + qs
                        zps = zpp.tile([P, VW], F32, tag='zps',
                                       name=f'zps_{qr}_{h}_{qs}')
                        for kt in range(qa + 1):
                            nc.tensor.matmul(zps[:],
                                             tiles[kt][:, qs * P:(qs + 1) * P],
                                             vres[:, kt, h * VW:(h + 1) * VW],
                                             start=(kt == 0), stop=(kt == qa))
                        rcp = zpool.tile([P, 1], F32, tag='rcp',
                                         name=f'rcp_{qr}_{h}_{qs}')
                        nc.vector.reciprocal(rcp[:], zps[:, P:P + 1])
                        zsb = zpool.tile([P, P], BF16, tag='zsb',
                                         name=f'zsb_{qr}_{h}_{qs}')
                        nc.vector.tensor_scalar_mul(zsb[:], zps[:, 0:P], rcp[:])
                        # delay the transpose one step so the DVE epilogue
                        # hides under the next PV block's matmuls
                        pend_t.append((h, qa, zsb))
                        if len(pend_t) > 1:
                            flush_t()

                def emit_s3(qr, ci):
                    ec, tl = divmod(ci, 4)
                    tt = 4 * qr + tl
                    es = slice(ec * 512, ec * 512 + 512)
                    pso = opp.tile([P, 512], F32, tag='pso', name=f'pso_{tt}_{ec}')
                    for h in range(HG):
                        nc.tensor.matmul(pso[:], zres[h][:, tt * P:(tt + 1) * P],
                                         wo_sb[:, ec, h, :],
                                         start=(h == 0), stop=(h == HG - 1))
                    osb = ospool.tile([P, 512], BF16, tag='osb',
                                      name=f'osb_{tt}_{ec}')
                    if (tt + ec) % 2 == 0:
                        nc.scalar.copy(osb[:], pso[:])
                    else:
                        nc.vector.tensor_copy(osb[:], pso[:])
                    nc.sync.dma_start(out[tt][:, es], osb[:])

                jobs = [(qr, h) for qr in range(NC512) for h in range(HG)]
                emit_qk(*jobs[0])
                for i, (qr, h) in enumerate(jobs):
                    if i + 1 < len(jobs):
                        emit_qk(*jobs[i + 1])
                    emit_pv(qr, h)
                    if qr >= 1:
                        emit_s3(qr - 1, 2 * h)
                        emit_s3(qr - 1, 2 * h + 1)
                while pend_t:
                    flush_t()
                for ci in range(16):
                    emit_s3(NC512 - 1, ci)

    nc.compile()
    return nc


def _host_inputs(x, Wqkv, Wo):
    """Build the 8 per-core input maps (all compute operands in bf16)."""
    BF = ml_dtypes.bfloat16
    # RoPE tables (f32 math, bf16 storage; sign folded into sinP rows 0:64)
    inv_freq = (1.0 / (BASE ** (np.arange(0, DH, 2, dtype=np.float32) / DH))).astype(np.float32)
    t = np.arange(S, dtype=np.float32)
    freqs = np.einsum('i,j->ij', t, inv_freq).astype(np.float32)   # [S, 64]
    emb = np.concatenate([freqs, freqs], axis=-1)                   # [S, 128]
    cosT = np.ascontiguousarray(np.cos(emb).T).astype(BF)           # [128, S]
    sinT = np.ascontiguousarray(np.sin(emb).T)
    sinT[0:64] = -sinT[0:64]
    sinP = sinT.astype(BF)

    # triangular causal mask [128, 128]: keep iff k_rel <= q_rel
    maskT = (np.arange(P)[:, None] <= np.arange(P)[None, :]).astype(BF)

    in_maps = []
    for c in range(8):
        b, g = c // 2, c % 2
        heads = range(HG * g, HG * g + HG)
        x_b = x[b]                                       # [S, D]
        # chunk-major [tcn, 128, 16, 512] so each chunk is one contiguous DMA
        xTc = np.ascontiguousarray(
            x_b.T.reshape(NDM, P, NC512, 512).transpose(2, 1, 0, 3)).astype(BF)
        # Q then K feature blocks, one per head in group
        blocks = [Wqkv[h * DH:(h + 1) * DH] for h in heads] + \
                 [Wqkv[D + h * DH:D + (h + 1) * DH] for h in heads]
        wqkT = np.stack([
            np.ascontiguousarray(
                blk.T.reshape(NDM, P, P).transpose(1, 0, 2)).astype(BF)
            for blk in blocks
        ])                                                       # [16, 128, 16, 128]
        Wv = np.concatenate([Wqkv[2 * D + h * DH:2 * D + (h + 1) * DH] for h in heads])
        # vc-major [vc, 128, 16, 256] so each vc block is one contiguous DMA
        wvT = np.ascontiguousarray(
            Wv.T.reshape(NDM, P, 4, 256).transpose(2, 1, 0, 3)).astype(BF)
        Wog = Wo[:, g * HG * DH:(g + 1) * HG * DH]               # [D, 1024]
        # ec-major [ec, 128, 8, 512] so each output chunk is one contiguous DMA
        woT = np.ascontiguousarray(
            Wog.T.reshape(HG, P, NC512, 512).transpose(2, 1, 0, 3)).astype(BF)
        in_maps.append({
            'xT': xTc, 'wqkT': wqkT, 'wvT': wvT, 'woT': woT,
            'cosT': cosT, 'sinP': sinP, 'maskT': maskT,
        })
    return in_maps


def kernel(x, Wqkv, Wo):
    from concourse.bass_utils import run_bass_kernel_spmd

    if 'nc' not in _CACHE:
        _CACHE['nc'] = _build_program()
    nc = _CACHE['nc']

    in_maps = _host_inputs(np.asarray(x, dtype=np.float32),
                           np.asarray(Wqkv, dtype=np.float32),
                           np.asarray(Wo, dtype=np.float32))
    res = run_bass_kernel_spmd(nc, in_maps, core_ids=list(range(8)))
    outs = [res.results[c]['out'].reshape(S, D).astype(np.float32) for c in range(8)]
    full = np.empty((B, S, D), dtype=np.float32)
    for b in range(B):
        full[b] = outs[2 * b] + outs[2 * b + 1]
    return full

